# revision 25
# baseline (speedup 1.0000x reference)
"""Trainium2 Bass kernel for nn_Attention_45930380263558.

EfficientViT-style attention with gathered relative position bias over
x:[16, 1024, 512]: qkv -> per-head softmax(q k^T * scale + bias) @ v -> proj.

Sharding: data-parallel over batch, 2 batches per core on 8 NeuronCores.

Key structure (v2, rewritten for PE warmth + engine balance):
  - Head-pair processing: heads 2p / 2p+1 live at partitions 0-63 / 64-127,
    so their K=64 S^T matmuls carry tile_position (0,0)/(64,0) and run
    CONCURRENTLY in the PE array (row tiling) when queue-adjacent.
  - P^T for a head pair is fully materialized in SBUF (pt buffer), so the
    PV + rowsum matmuls of pair p-1 fill the PE queue while ScalarE exps
    pair p -> no PE dependency bubbles, HAM stays at K=8/8.
  - exp as [128,1024] ACTs reading 2-bank PSUM tiles (halves ACT overhead).
  - Rowsums for all (head, q-half) accumulate into ONE [16,512] PSUM bank
    via a sliding ones-column selector (value 1/256); a single VectorE
    reciprocal per batch replaces 16 pathological 4us reciprocals.
    (1/256 scale un-done by scaling proj_w by 1/256 on the host.)
  - Normalization deferred: unnormalized O^T cast to fp16 (pin); inv
    broadcast via small DRAM roundtrip; normalize + proj of batch b run
    inside batch b+1's QKV/score window.
  - Last head pair streams its PV/rowsum inside its own score phase
    (1-iteration lag) so the batch tail drains densely.
  - QKV copies on ScalarE, attention elementwise on VectorE (balance).

PSUM banks: st 2x[128,1024]=4, o 2x[128,512]=2, rs [16,512]=1,
fl [128,512]=1 (v-halves / proj tiles)  -> 8 total.
"""

import os
import sys

for _p in ("/opt/trn_rl_repo",):
    if _p not in sys.path and os.path.isdir(_p):
        sys.path.insert(0, _p)

from contextlib import ExitStack

import numpy as np

import concourse.bass as bass
import concourse.tile as tile
from concourse import bacc, mybir
from concourse.bass_utils import run_bass_kernel_spmd

F32 = mybir.dt.float32
F16 = mybir.dt.float16
BF16 = mybir.dt.bfloat16
F8 = mybir.dt.float8e4

N_CORES = 8
B = 16
B_LOC = B // N_CORES  # 2
N = 1024  # tokens
D = 512  # model dim
H = 8  # heads
DK = 64  # key dim
DV = 128  # value dim per head
SCALE = DK ** -0.5
NT = N // 128  # 8 token tiles
DC = D // 128  # 4 dim chunks
QH = 2  # q halves of 512
NPAIR = H // 2  # 4 head pairs
RS_SCALE = 1.0 / 256.0  # rowsum pre-scale, undone in proj_w

LAST_RESULT = None


def _ensure_axon_hooks_module():
    try:
        import antenv.axon_hooks  # noqa: F401
        return
    except ImportError:
        pass
    import types

    import antenv

    m = types.ModuleType("antenv.axon_hooks")
    m._hook = None

    def set_axon_ntff_profile_hook(h):
        m._hook = h

    def get_axon_ntff_profile_hook():
        return m._hook

    m.set_axon_ntff_profile_hook = set_axon_ntff_profile_hook
    m.get_axon_ntff_profile_hook = get_axon_ntff_profile_hook
    sys.modules["antenv.axon_hooks"] = m
    antenv.axon_hooks = m


_ensure_axon_hooks_module()


def build_program(use_qkv_bias: bool, use_proj_bias: bool):
    nc = bacc.Bacc("TRN2", target_bir_lowering=False, debug=False,
                   num_devices=N_CORES)

    xT_d = nc.dram_tensor("xT", [B_LOC, DC, 128, N], F16, kind="ExternalInput").ap()
    w_qk_d = nc.dram_tensor("w_qk", [DC, 128, N], F16, kind="ExternalInput").ap()
    w_v_d = nc.dram_tensor("w_v", [DC, 128, N], F16, kind="ExternalInput").ap()
    bias_d = nc.dram_tensor("bias", [H, NT, 128, N], F16, kind="ExternalInput").ap()
    w_proj_d = nc.dram_tensor("w_proj", [H, 128, D], F16, kind="ExternalInput").ap()
    selb_d = nc.dram_tensor("selb", [128, 31], F16, kind="ExternalInput").ap()
    ones_d = nc.dram_tensor("ones", [128, N], F16, kind="ExternalInput").ap()
    inv_scr = nc.dram_tensor("inv_scratch", [B_LOC, 16, 512], F16).ap()
    out_d = nc.dram_tensor("out", [B_LOC, N, D], F32, kind="ExternalOutput").ap()
    if use_qkv_bias:
        qk_bias_d = nc.dram_tensor("qk_bias", [1, N], F16, kind="ExternalInput").ap()
        v_bias_d = nc.dram_tensor("v_bias", [1, N], F16, kind="ExternalInput").ap()
    if use_proj_bias:
        proj_bias_d = nc.dram_tensor("proj_bias", [1, D], F16, kind="ExternalInput").ap()

    with tile.TileContext(nc) as tc, ExitStack() as ctx:
        consts = ctx.enter_context(tc.tile_pool(name="consts", bufs=1))
        xp = ctx.enter_context(tc.tile_pool(name="xp", bufs=2))
        qkp = ctx.enter_context(tc.tile_pool(name="qkp", bufs=1))
        vp = ctx.enter_context(tc.tile_pool(name="vp", bufs=1))
        biasp = ctx.enter_context(tc.tile_pool(name="biasp", bufs=3))
        ptp = ctx.enter_context(tc.tile_pool(name="ptp", bufs=2))
        ep = ctx.enter_context(tc.tile_pool(name="ep", bufs=2))
        pinp = ctx.enter_context(tc.tile_pool(name="pinp", bufs=1))
        bcrp = ctx.enter_context(tc.tile_pool(name="bcrp", bufs=2))
        invp = ctx.enter_context(tc.tile_pool(name="invp", bufs=1))
        outp = ctx.enter_context(tc.tile_pool(name="outp", bufs=2))

        ps_st = ctx.enter_context(tc.tile_pool(name="ps_st", bufs=2, space="PSUM"))
        ps_o = ctx.enter_context(tc.tile_pool(name="ps_o", bufs=2, space="PSUM"))
        ps_rs = ctx.enter_context(tc.tile_pool(name="ps_rs", bufs=1, space="PSUM"))
        ps_fl = ctx.enter_context(tc.tile_pool(name="ps_fl", bufs=1, space="PSUM"))

        # ---- constants ----
        w_qk_t = consts.tile([128, DC, N], F16)
        w_v_t = consts.tile([128, DC, N], F16)
        for kc in range(DC):
            for ps in range(2):
                sl = slice(64 * ps, 64 * (ps + 1))
                nc.sync.dma_start(out=w_qk_t[sl, kc, :], in_=w_qk_d[kc, sl, :])
                nc.sync.dma_start(out=w_v_t[sl, kc, :], in_=w_v_d[kc, sl, :])
        w_proj_t = consts.tile([128, H, D], F16)
        for ps in range(4):
            sl = slice(32 * ps, 32 * (ps + 1))
            nc.sync.dma_start(out=w_proj_t[sl, :, :],
                              in_=w_proj_d.transpose([1, 0, 2])[sl, :, :])
        selb_t = consts.tile([128, 31], F16)
        nc.sync.dma_start(out=selb_t, in_=selb_d)
        ones_t = consts.tile([128, N], F16)
        nc.sync.dma_start(out=ones_t, in_=ones_d)
        ones_row = ones_t[0:1, 0:128]
        if use_qkv_bias:
            qk_bias_t = consts.tile([1, N], F16)
            nc.sync.dma_start(out=qk_bias_t, in_=qk_bias_d)
            v_bias_t = consts.tile([1, N], F16)
            nc.sync.dma_start(out=v_bias_t, in_=v_bias_d)
            ones_n = ones_t[0:1, :]
        if use_proj_bias:
            proj_bias_t = consts.tile([1, D], F16)
            nc.sync.dma_start(out=proj_bias_t, in_=proj_bias_d)

        # ---- x prefetch (both batches early) ----
        x_ts = []
        for b in range(B_LOC):
            x_t = xp.tile([128, DC, N], F16, name=f"x_t{b}")
            for kc in range(DC):
                for ps in range(2):
                    sl = slice(64 * ps, 64 * (ps + 1))
                    nc.sync.dma_start(out=x_t[sl, kc, :], in_=xT_d[b, kc, sl, :])
            x_ts.append(x_t)

        # =============== emission helpers ===============

        def emit_qk_tile(x_t, qk_sb, mt):
            """One qk m-tile: [128ch, 1024tok] psum, cast on ScalarE."""
            st = ps_st.tile([128, N], F32, tag="st", name="qkps")
            for nt in range(QH):
                for kc in range(DC):
                    nc.tensor.matmul(
                        st[:, nt * 512:(nt + 1) * 512],
                        lhsT=w_qk_t[:, kc, mt * 128:(mt + 1) * 128],
                        rhs=x_t[:, kc, nt * 512:(nt + 1) * 512],
                        start=(kc == 0),
                        stop=(kc == DC - 1 and not use_qkv_bias),
                    )
                if use_qkv_bias:
                    nc.tensor.matmul(
                        st[:, nt * 512:(nt + 1) * 512],
                        lhsT=qk_bias_t[:, mt * 128:(mt + 1) * 128],
                        rhs=ones_n[:, nt * 512:(nt + 1) * 512],
                        start=False, stop=True,
                    )
            with nc.allow_low_precision(reason="fp16 activations"):
                nc.scalar.copy(qk_sb[:, mt, :], st)

        def emit_v_half(x_t, v_sb, tt, nh):
            """Half v token-tile: [128tok, 512vch] psum (fl), cast ScalarE."""
            fl = ps_fl.tile([128, 512], F32, tag="fl", name="vps")
            for kc in range(DC):
                nc.tensor.matmul(
                    fl,
                    lhsT=x_t[:, kc, tt * 128:(tt + 1) * 128],
                    rhs=w_v_t[:, kc, nh * 512:(nh + 1) * 512],
                    start=(kc == 0),
                    stop=(kc == DC - 1 and not use_qkv_bias),
                )
            if use_qkv_bias:
                nc.tensor.matmul(
                    fl,
                    lhsT=ones_n[:, tt * 128:(tt + 1) * 128],
                    rhs=v_bias_t[:, nh * 512:(nh + 1) * 512],
                    start=False, stop=True,
                )
            with nc.allow_low_precision(reason="fp16 activations"):
                # alternate engines: ScalarE is exp-critical during A(0)
                if (tt + nh) % 2 == 0:
                    nc.scalar.copy(v_sb[:, tt, nh * 512:(nh + 1) * 512], fl)
                else:
                    nc.vector.tensor_copy(v_sb[:, tt, nh * 512:(nh + 1) * 512],
                                          fl)

        def b_phase_closures(v_sb, pt_buf, pin, rs_ps, p, rs_first, rs_last,
                             split_rs=False):
            """PV + rowsum + o-cast closures for head pair p (reads pt_buf).

            split_rs: return (main_ops, rs_ops) so the caller can front-load
            all rowsum matmuls (early reciprocal in the batch tail).
            """
            ops = []
            rs_ops = []
            for hp in range(2):
                h = 2 * p + hp
                o_tiles = {}

                def mk_alloc(qh, o_tiles=o_tiles):
                    def alloc():
                        o_tiles[qh] = ps_o.tile([128, 512], F32, tag="o",
                                                name="o_ps")
                    return alloc

                def mk_pv(qh, kc, hp=hp, h=h, o_tiles=o_tiles):
                    def pv():
                        nc.tensor.matmul(
                            o_tiles[qh],
                            lhsT=v_sb[:, kc, h * 128:(h + 1) * 128],
                            rhs=pt_buf[:, hp, kc, qh * 512:(qh + 1) * 512],
                            start=(kc == 0), stop=(kc == NT - 1),
                        )
                    return pv

                def mk_cast(qh, hp=hp, h=h, o_tiles=o_tiles):
                    def cast_o():
                        with nc.allow_low_precision(reason="fp16 unnorm O^T"):
                            nc.vector.tensor_copy(
                                pin[:, h, qh * 512:(qh + 1) * 512], o_tiles[qh])
                    return cast_o

                def mk_rs(qh, kc, hp=hp, h=h):
                    j = 2 * h + qh
                    first = rs_first and (hp == 0 and qh == 0 and kc == 0)
                    last = rs_last and (hp == 1 and qh == 1 and kc == NT - 1)

                    def rs():
                        nc.tensor.matmul(
                            rs_ps,
                            lhsT=selb_t[:, 15 - j:31 - j],
                            rhs=pt_buf[:, hp, kc, qh * 512:(qh + 1) * 512],
                            start=first, stop=last,
                            skip_group_check=True,
                        )
                    return rs

                ops.append(mk_alloc(0))
                ops.append(mk_alloc(1))
                tgt = rs_ops if split_rs else ops
                for kc in range(NT):
                    # qh-paired PVs share the V-chunk stationary operand
                    ops.append(mk_pv(0, kc))
                    ops.append(mk_pv(1, kc))
                    tgt.append(mk_rs(0, kc))
                    tgt.append(mk_rs(1, kc))
                ops.append(mk_cast(0))
                ops.append(mk_cast(1))
            if split_rs:
                return ops, rs_ops
            return ops

        def load_bias_block(bias_blocks, p, blk):
            """Load bias block blk (2 kc-tiles) for head pair p; 4 DMAs."""
            bb = bias_blocks[p][blk]
            if "t" in bb:
                return
            t = biasp.tile([128, 2, 2, N], F16, name="bias_t")
            for hp in range(2):
                h = 2 * p + hp
                for kcm in range(2):
                    for ps in range(2):
                        nc.sync.dma_start(
                            out=t[64 * ps:64 * (ps + 1), hp, kcm, :],
                            in_=bias_d[h, 2 * blk + kcm, 64 * ps:64 * (ps + 1), :],
                        )
            bb["t"] = t

        def emit_a_phase(qk_sb, pt_buf, bias_blocks, p, fillers,
                         lagged=None, fill_iters=NT):
            """Score phase for head pair p: row-tiled S^T -> exp -> bias-mult.

            Emission order per kc: S^T MMs, exp+mult, then filler closures
            (PE work that does NOT touch the st tag), then bias prefetch.
            fillers are spread over the first fill_iters iterations; lagged
            maps kc -> closures emitted at that iteration (tail-pair PV).
            """
            n_fill = len(fillers)
            fi = 0
            for kc in range(NT):
                sts = [ps_st.tile([128, N], F32, tag="st", name="st_ps")
                       for _ in range(2)]
                for hp in range(2):
                    for qh in range(QH):
                        h = 2 * p + hp
                        par = (h % 2) * 64
                        nc.tensor.matmul(
                            sts[hp][:, qh * 512:(qh + 1) * 512],
                            lhsT=qk_sb[par:par + 64, 4 + h // 2,
                                       kc * 128:(kc + 1) * 128],
                            rhs=qk_sb[par:par + 64, h // 2,
                                      qh * 512:(qh + 1) * 512],
                            start=True, stop=True,
                        )
                for hp in range(2):
                    e_t = ep.tile([128, N], F16, name="e_t")
                    with nc.allow_low_precision(reason="fp16 exp"):
                        nc.scalar.activation(
                            e_t, sts[hp],
                            mybir.ActivationFunctionType.Exp, scale=SCALE)
                    with nc.allow_low_precision(reason="fp16 P^T"):
                        nc.vector.tensor_tensor(
                            pt_buf[:, hp, kc, :], e_t,
                            bias_blocks[p][kc // 2]["t"][:, hp, kc % 2, :],
                            op=mybir.AluOpType.mult)
                ki = min(kc + 1, fill_iters)
                take = (n_fill * ki) // fill_iters - (n_fill * min(kc, fill_iters)) // fill_iters
                for _ in range(take):
                    fillers[fi]()
                    fi += 1
                if lagged is not None:
                    for op in lagged.get(kc, ()):
                        op()
                # alternating bias prefetch, 4-iteration lead: b2@kc0,
                # b3@kc2, next pair's b0@kc4, b1@kc6
                if kc % 2 == 0:
                    blk = kc // 2 + 2
                    if blk < NT // 2:
                        load_bias_block(bias_blocks, p, blk)
                    elif p + 1 < NPAIR:
                        load_bias_block(bias_blocks, p + 1, blk - NT // 2)
            assert fi == n_fill, (fi, n_fill)

        def tail_pair_ops(b, v_sb, pt_buf, pin, rs_ps, p, rs_last):
            """Last head pair of a batch: h_even streams inside A(p) via
            `lagged` (1-iteration lag behind the bias-mults); h_odd + its
            rowsums + recip + remaining casts drain densely afterwards."""
            lagged = {}
            post = []
            o_tiles = {}

            def mk_pv(hp, qh, kc):
                h = 2 * p + hp

                def pv():
                    nc.tensor.matmul(
                        o_tiles[(hp, qh)],
                        lhsT=v_sb[:, kc, h * 128:(h + 1) * 128],
                        rhs=pt_buf[:, hp, kc, qh * 512:(qh + 1) * 512],
                        start=(kc == 0), stop=(kc == NT - 1),
                    )
                return pv

            def mk_rs(hp, qh, kc):
                h = 2 * p + hp
                j = 2 * h + qh
                last = rs_last and (hp == 1 and qh == 1 and kc == NT - 1)

                def rs():
                    nc.tensor.matmul(
                        rs_ps,
                        lhsT=selb_t[:, 15 - j:31 - j],
                        rhs=pt_buf[:, hp, kc, qh * 512:(qh + 1) * 512],
                        start=False, stop=last,
                        skip_group_check=True,
                    )
                return rs

            def mk_cast(hp, qh):
                h = 2 * p + hp

                def cast_o():
                    with nc.allow_low_precision(reason="fp16 unnorm O^T"):
                        nc.vector.tensor_copy(
                            pin[:, h, qh * 512:(qh + 1) * 512],
                            o_tiles[(hp, qh)])
                return cast_o

            def mk_alloc(hp, qh):
                def alloc():
                    o_tiles[(hp, qh)] = ps_o.tile([128, 512], F32, tag="o",
                                                  name="o_ps")
                return alloc

            # h_even (hp=0): both q-halves stream per kc, 1-iteration lag,
            # starting at iteration 4 (B(p-1) fillers occupy iters 0-3)
            for kc in range(NT):
                it = max(kc + 1, 4)
                ops = lagged.setdefault(min(it, NT - 1), [])
                if kc == 0:
                    ops.append(mk_alloc(0, 0))
                    ops.append(mk_alloc(0, 1))
                ops.append(mk_pv(0, 0, kc))
                ops.append(mk_pv(0, 1, kc))
                ops.append(mk_rs(0, 0, kc))
                ops.append(mk_rs(0, 1, kc))
            # post: finish h_even casts, then h_odd rowsums (-> early recip
            # overlapping h_odd PV), then h_odd PV groups + casts
            post.append(mk_cast(0, 0))
            post.append(mk_cast(0, 1))
            for kc in range(NT):
                post.append(mk_rs(1, 0, kc))
                post.append(mk_rs(1, 1, kc))
            post.append(lambda: emit_recip_chain(b, rs_ps))
            post.append(mk_alloc(1, 0))
            post.append(mk_alloc(1, 1))
            for kc in range(NT):
                post.append(mk_pv(1, 0, kc))
                post.append(mk_pv(1, 1, kc))
            post.append(mk_cast(1, 0))
            post.append(mk_cast(1, 1))
            return lagged, post

        def emit_recip_chain(b, rs_ps):
            inv_t = invp.tile([16, 512], F16, name="inv_t")
            with nc.allow_low_precision(reason="fp16 softmax inv-denominators"):
                nc.vector.reciprocal(inv_t, rs_ps)
            nc.sync.dma_start(out=inv_scr[b], in_=inv_t)

        def norm_closures(b, pin):
            """Normalize O^T by inv denominators. The [1,1024] inv row is
            fetched to partition 0 (2KB DMA) and broadcast across the 128
            partitions with a K=1 ones-column matmul into PSUM -- no bulk
            DMA, and the PE stays warm through the epilogue."""
            ops = []

            def bcast_and_norm(h):
                bcrow = bcrp.tile([1, N], F16, name="bcrow")
                row_src = bass.AP(
                    tensor=inv_scr.tensor,
                    offset=inv_scr.offset + (b * 16 * 512) + h * N,
                    ap=[[0, 1], [1, N]],
                )
                nc.sync.dma_start(out=bcrow, in_=row_src)
                for qh in range(QH):
                    bc_ps = ps_o.tile([128, 512], F32, tag="o", name="bc_ps")
                    nc.tensor.matmul(
                        bc_ps,
                        lhsT=ones_t[0:1, 0:128],
                        rhs=bcrow[0:1, qh * 512:(qh + 1) * 512],
                        start=True, stop=True,
                    )
                    with nc.allow_low_precision(reason="fp16 normalized O^T"):
                        nc.vector.tensor_tensor(
                            pin[:, h, qh * 512:(qh + 1) * 512],
                            pin[:, h, qh * 512:(qh + 1) * 512],
                            bc_ps, op=mybir.AluOpType.mult)

            for h in range(H):
                ops.append(lambda h=h: bcast_and_norm(h))
            return ops

        def proj_mm_closures(b, pin):
            """Proj matmuls + out DMA (needs all norms done)."""
            ops = []
            for qt in range(NT):
                pp_tile = {}

                def alloc_pp(pp_tile=pp_tile):
                    pp_tile["t"] = ps_fl.tile([128, 512], F32, tag="fl",
                                              name="pp_ps")

                def proj_mms(qt=qt, pp_tile=pp_tile):
                    pp = pp_tile["t"]
                    for h in range(H):
                        last = (h == H - 1)
                        nc.tensor.matmul(
                            pp,
                            lhsT=pin[:, h, qt * 128:(qt + 1) * 128],
                            rhs=w_proj_t[:, h, :],
                            start=(h == 0),
                            stop=(last and not use_proj_bias),
                        )
                    if use_proj_bias:
                        nc.tensor.matmul(
                            pp,
                            lhsT=ones_row,
                            rhs=proj_bias_t,
                            start=False, stop=True,
                        )

                def flush_pp(qt=qt, pp_tile=pp_tile):
                    ot = outp.tile([128, 512], F32, name="ot")
                    nc.vector.tensor_copy(ot, pp_tile["t"])
                    # 4 row-split DMAs (contiguous 2KB rows, 4 queues)
                    for rp in range(4):
                        nc.sync.dma_start(
                            out=out_d[b, qt * 128 + rp * 32:
                                      qt * 128 + (rp + 1) * 32, :],
                            in_=ot[rp * 32:(rp + 1) * 32, :],
                        )

                ops.append(alloc_pp)
                ops.append(proj_mms)
                ops.append(flush_pp)
            return ops

        # =============== main emission ===============

        prev_tail = []  # closures pending from previous batch (B3 + recip)
        prev_pin = None
        for b in range(B_LOC):
            x_t = x_ts[b]
            qk_sb = qkp.tile([128, NT, N], F16, name="qk_sb")
            v_sb = vp.tile([128, NT, N], F16, name="v_sb")
            pin = pinp.tile([128, H, N], F16, name="pin")
            rs_ps = ps_rs.tile([16, 512], F32, tag="rs", name="rs_ps")
            pt_bufs = [ptp.tile([128, 2, NT, N], F16, name=f"pt{i}",
                                tag=f"pt{i}", bufs=1)
                       for i in range(2)]
            bias_blocks = [[{} for _ in range(NT // 2)] for _ in range(NPAIR)]

            # qk m-tiles (pair-0 deps first), zipped with prev batch tail
            qk_order = [0, 4, 1, 5, 2, 6, 3, 7]
            nqk = len(qk_order)
            ntail = len(prev_tail)
            ti = 0
            for i, mt in enumerate(qk_order):
                emit_qk_tile(x_t, qk_sb, mt)
                take = (ntail * (i + 1)) // nqk - (ntail * i) // nqk
                for _ in range(take):
                    prev_tail[ti]()
                    ti += 1
            assert ti == ntail
            prev_tail = []

            load_bias_block(bias_blocks, 0, 0)
            load_bias_block(bias_blocks, 0, 1)

            # A(0) fillers: v-halves (PE) + norms of b-1 (Vector/DMA)
            fillers = [
                (lambda tt=tt, nh=nh: emit_v_half(x_t, v_sb, tt, nh))
                for tt in range(NT) for nh in range(QH)
            ]
            if b > 0:
                fillers = fillers + norm_closures(b - 1, prev_pin)
            emit_a_phase(qk_sb, pt_bufs[0], bias_blocks, 0, fillers)

            for p in range(1, NPAIR):
                bops = b_phase_closures(
                    v_sb, pt_bufs[(p - 1) % 2], pin, rs_ps, p - 1,
                    rs_first=(p - 1 == 0), rs_last=False)
                if p == 1 and b > 0:
                    # proj(b-1) MMs read pin(b-1): MUST precede the o-casts
                    # of batch b (which overwrite the pin slot)
                    bops = proj_mm_closures(b - 1, prev_pin) + bops
                if p == NPAIR - 1:
                    lagged, tail_post = tail_pair_ops(
                        b, v_sb, pt_bufs[p % 2], pin, rs_ps, p, rs_last=True)
                    emit_a_phase(qk_sb, pt_bufs[p % 2], bias_blocks, p, bops,
                                 lagged=lagged, fill_iters=4)
                else:
                    emit_a_phase(qk_sb, pt_bufs[p % 2], bias_blocks, p, bops)

            prev_tail = tail_post
            prev_pin = pin

        # final tail: B3(last), recip(last), proj(last)
        for op in prev_tail:
            op()
        for op in norm_closures(B_LOC - 1, prev_pin):
            op()
        for op in proj_mm_closures(B_LOC - 1, prev_pin):
            op()

    nc.compile()
    return nc


def _prep_core_inputs(x, qkv_w, qkv_b, proj_w, proj_b, attn_biases, bias_idxs):
    """Host-side layout preparation. Returns (shared, per_core_xT, flags)."""
    x = np.ascontiguousarray(np.asarray(x, np.float32))
    qkv_w = np.asarray(qkv_w, np.float32)
    qkv_b = np.asarray(qkv_b, np.float32)
    proj_w = np.asarray(proj_w, np.float32)
    proj_b = np.asarray(proj_b, np.float32)
    attn_biases = np.asarray(attn_biases, np.float32)
    bias_idxs = np.asarray(bias_idxs)

    # qkv_w columns: per head 256 = [q 64 | k 64 | v 128]
    Wh = qkv_w.reshape(D, H, 256)
    w_q = Wh[:, :, :DK].reshape(D, H * DK)
    w_k = Wh[:, :, DK:2 * DK].reshape(D, H * DK)
    w_qk = np.concatenate([w_q, w_k], axis=1)          # [512, 1024]
    w_v = Wh[:, :, 2 * DK:].reshape(D, H * DV)         # [512, 1024]

    bh = qkv_b.reshape(H, 256)
    qk_bias = np.concatenate([bh[:, :DK].reshape(-1), bh[:, DK:2 * DK].reshape(-1)])
    v_bias = bh[:, 2 * DK:].reshape(-1)

    # exp of gathered bias, transposed to [H, k, q], tiled [H, NT, 128, N]
    BT = np.ascontiguousarray(
        np.exp(attn_biases[:, bias_idxs]).transpose(0, 2, 1))
    bias = BT.reshape(H, NT, 128, N).astype(np.float16)

    # sliding ones-column selector: col 15 = RS_SCALE, rest 0
    selb = np.zeros((128, 31), np.float16)
    selb[:, 15] = RS_SCALE

    shared = {
        "ones": np.ones((128, N), np.float16),
        "selb": selb,
        "w_qk": np.ascontiguousarray(w_qk.reshape(DC, 128, H * DK * 2)).astype(np.float16),
        "w_v": np.ascontiguousarray(w_v.reshape(DC, 128, H * DV)).astype(np.float16),
        "bias": bias,
        # 1/256 here cancels the RS_SCALE in the rowsum selector
        "w_proj": np.ascontiguousarray(
            (proj_w * (1.0 / 256.0)).reshape(H, 128, D)).astype(np.float16),
    }
    use_qkv_bias = bool(np.any(qkv_b))
    use_proj_bias = bool(np.any(proj_b))
    if use_qkv_bias:
        shared["qk_bias"] = qk_bias.reshape(1, N).astype(np.float16)
        shared["v_bias"] = v_bias.reshape(1, N).astype(np.float16)
    if use_proj_bias:
        shared["proj_bias"] = proj_b.reshape(1, D).astype(np.float16)

    xT = np.ascontiguousarray(x.transpose(0, 2, 1)).reshape(B, DC, 128, N)
    xT = xT.astype(np.float16)
    per_core = [xT[c * B_LOC:(c + 1) * B_LOC] for c in range(N_CORES)]
    return shared, per_core, use_qkv_bias, use_proj_bias


def kernel(x, qkv_w, qkv_b, proj_w, proj_b, attn_biases, bias_idxs):
    global LAST_RESULT
    shared, per_core, use_qkv_bias, use_proj_bias = _prep_core_inputs(
        x, qkv_w, qkv_b, proj_w, proj_b, attn_biases, bias_idxs)

    nc = build_program(use_qkv_bias, use_proj_bias)

    in_maps = [dict(shared, xT=per_core[c]) for c in range(N_CORES)]
    trace = bool(os.environ.get("BASS_TRACE"))
    res = run_bass_kernel_spmd(nc, in_maps, core_ids=list(range(N_CORES)),
                               trace=trace)
    LAST_RESULT = res
    out = np.concatenate([res.results[c]["out"] for c in range(N_CORES)], axis=0)
    return np.ascontiguousarray(out.astype(np.float32))


# revision 26
# speedup vs baseline: 1.0239x; 1.0239x over previous
"""Trainium2 Bass kernel for nn_Attention_45930380263558.

EfficientViT-style attention with gathered relative position bias over
x:[16, 1024, 512]: qkv -> per-head softmax(q k^T * scale + bias) @ v -> proj.

Sharding: data-parallel over batch, 2 batches per core on 8 NeuronCores.

Key structure (v2, rewritten for PE warmth + engine balance):
  - Head-pair processing: heads 2p / 2p+1 live at partitions 0-63 / 64-127,
    so their K=64 S^T matmuls carry tile_position (0,0)/(64,0) and run
    CONCURRENTLY in the PE array (row tiling) when queue-adjacent.
  - P^T for a head pair is fully materialized in SBUF (pt buffer), so the
    PV + rowsum matmuls of pair p-1 fill the PE queue while ScalarE exps
    pair p -> no PE dependency bubbles, HAM stays at K=8/8.
  - exp as [128,1024] ACTs reading 2-bank PSUM tiles (halves ACT overhead).
  - Rowsums for all (head, q-half) accumulate into ONE [16,512] PSUM bank
    via a sliding ones-column selector (value 1/256); a single VectorE
    reciprocal per batch replaces 16 pathological 4us reciprocals.
    (1/256 scale un-done by scaling proj_w by 1/256 on the host.)
  - Normalization deferred: unnormalized O^T cast to fp16 (pin); inv
    broadcast via small DRAM roundtrip; normalize + proj of batch b run
    inside batch b+1's QKV/score window.
  - Last head pair streams its PV/rowsum inside its own score phase
    (1-iteration lag) so the batch tail drains densely.
  - QKV copies on ScalarE, attention elementwise on VectorE (balance).

PSUM banks: st 2x[128,1024]=4, o 2x[128,512]=2, rs [16,512]=1,
fl [128,512]=1 (v-halves / proj tiles)  -> 8 total.
"""

import os
import sys

for _p in ("/opt/trn_rl_repo",):
    if _p not in sys.path and os.path.isdir(_p):
        sys.path.insert(0, _p)

from contextlib import ExitStack

import numpy as np

import concourse.bass as bass
import concourse.tile as tile
from concourse import bacc, mybir
from concourse.bass_utils import run_bass_kernel_spmd

F32 = mybir.dt.float32
F16 = mybir.dt.float16
BF16 = mybir.dt.bfloat16
F8 = mybir.dt.float8e4

N_CORES = 8
B = 16
B_LOC = B // N_CORES  # 2
N = 1024  # tokens
D = 512  # model dim
H = 8  # heads
DK = 64  # key dim
DV = 128  # value dim per head
SCALE = DK ** -0.5
NT = N // 128  # 8 token tiles
DC = D // 128  # 4 dim chunks
QH = 2  # q halves of 512
NPAIR = H // 2  # 4 head pairs
RS_SCALE = 1.0 / 256.0  # rowsum pre-scale, undone in proj_w

LAST_RESULT = None


def _ensure_axon_hooks_module():
    try:
        import antenv.axon_hooks  # noqa: F401
        return
    except ImportError:
        pass
    import types

    import antenv

    m = types.ModuleType("antenv.axon_hooks")
    m._hook = None

    def set_axon_ntff_profile_hook(h):
        m._hook = h

    def get_axon_ntff_profile_hook():
        return m._hook

    m.set_axon_ntff_profile_hook = set_axon_ntff_profile_hook
    m.get_axon_ntff_profile_hook = get_axon_ntff_profile_hook
    sys.modules["antenv.axon_hooks"] = m
    antenv.axon_hooks = m


_ensure_axon_hooks_module()


def build_program(use_qkv_bias: bool, use_proj_bias: bool):
    nc = bacc.Bacc("TRN2", target_bir_lowering=False, debug=False,
                   num_devices=N_CORES)

    xT_d = nc.dram_tensor("xT", [B_LOC, DC, 128, N], F16, kind="ExternalInput").ap()
    w_qk_d = nc.dram_tensor("w_qk", [DC, 128, N], F16, kind="ExternalInput").ap()
    w_v_d = nc.dram_tensor("w_v", [DC, 128, N], F16, kind="ExternalInput").ap()
    bias_d = nc.dram_tensor("bias", [H, NT, 128, N], F16, kind="ExternalInput").ap()
    w_proj_d = nc.dram_tensor("w_proj", [H, 128, D], F16, kind="ExternalInput").ap()
    selb_d = nc.dram_tensor("selb", [128, 31], F16, kind="ExternalInput").ap()
    ones_d = nc.dram_tensor("ones", [128, N], F16, kind="ExternalInput").ap()
    inv_scr = nc.dram_tensor("inv_scratch", [B_LOC, 16, 512], F16).ap()
    out_d = nc.dram_tensor("out", [B_LOC, N, D], F32, kind="ExternalOutput").ap()
    if use_qkv_bias:
        qk_bias_d = nc.dram_tensor("qk_bias", [1, N], F16, kind="ExternalInput").ap()
        v_bias_d = nc.dram_tensor("v_bias", [1, N], F16, kind="ExternalInput").ap()
    if use_proj_bias:
        proj_bias_d = nc.dram_tensor("proj_bias", [1, D], F16, kind="ExternalInput").ap()

    with tile.TileContext(nc) as tc, ExitStack() as ctx:
        consts = ctx.enter_context(tc.tile_pool(name="consts", bufs=1))
        xp = ctx.enter_context(tc.tile_pool(name="xp", bufs=2))
        qkp = ctx.enter_context(tc.tile_pool(name="qkp", bufs=1))
        vp = ctx.enter_context(tc.tile_pool(name="vp", bufs=1))
        biasp = ctx.enter_context(tc.tile_pool(name="biasp", bufs=3))
        ptp = ctx.enter_context(tc.tile_pool(name="ptp", bufs=2))
        ep = ctx.enter_context(tc.tile_pool(name="ep", bufs=2))
        pinp = ctx.enter_context(tc.tile_pool(name="pinp", bufs=1))
        bcrp = ctx.enter_context(tc.tile_pool(name="bcrp", bufs=2))
        invp = ctx.enter_context(tc.tile_pool(name="invp", bufs=1))
        outp = ctx.enter_context(tc.tile_pool(name="outp", bufs=2))

        ps_st = ctx.enter_context(tc.tile_pool(name="ps_st", bufs=2, space="PSUM"))
        ps_o = ctx.enter_context(tc.tile_pool(name="ps_o", bufs=2, space="PSUM"))
        ps_rs = ctx.enter_context(tc.tile_pool(name="ps_rs", bufs=1, space="PSUM"))
        ps_fl = ctx.enter_context(tc.tile_pool(name="ps_fl", bufs=1, space="PSUM"))

        # ---- constants ----
        w_qk_t = consts.tile([128, DC, N], F16)
        w_v_t = consts.tile([128, DC, N], F16)
        for kc in range(DC):
            for ps in range(2):
                sl = slice(64 * ps, 64 * (ps + 1))
                nc.sync.dma_start(out=w_qk_t[sl, kc, :], in_=w_qk_d[kc, sl, :])
                nc.sync.dma_start(out=w_v_t[sl, kc, :], in_=w_v_d[kc, sl, :])
        w_proj_t = consts.tile([128, H, D], F16)
        for ps in range(4):
            sl = slice(32 * ps, 32 * (ps + 1))
            nc.sync.dma_start(out=w_proj_t[sl, :, :],
                              in_=w_proj_d.transpose([1, 0, 2])[sl, :, :])
        selb_t = consts.tile([128, 31], F16)
        nc.sync.dma_start(out=selb_t, in_=selb_d)
        ones_t = consts.tile([128, N], F16)
        nc.sync.dma_start(out=ones_t, in_=ones_d)
        ones_row = ones_t[0:1, 0:128]
        if use_qkv_bias:
            qk_bias_t = consts.tile([1, N], F16)
            nc.sync.dma_start(out=qk_bias_t, in_=qk_bias_d)
            v_bias_t = consts.tile([1, N], F16)
            nc.sync.dma_start(out=v_bias_t, in_=v_bias_d)
            ones_n = ones_t[0:1, :]
        if use_proj_bias:
            proj_bias_t = consts.tile([1, D], F16)
            nc.sync.dma_start(out=proj_bias_t, in_=proj_bias_d)

        # ---- x prefetch (both batches early) ----
        x_ts = []
        for b in range(B_LOC):
            x_t = xp.tile([128, DC, N], F16, name=f"x_t{b}")
            for kc in range(DC):
                for ps in range(2):
                    sl = slice(64 * ps, 64 * (ps + 1))
                    nc.sync.dma_start(out=x_t[sl, kc, :], in_=xT_d[b, kc, sl, :])
            x_ts.append(x_t)

        # =============== emission helpers ===============

        def emit_qk_tile(x_t, qk_sb, mt):
            """One qk m-tile: [128ch, 1024tok] psum, cast on ScalarE."""
            st = ps_st.tile([128, N], F32, tag="st", name="qkps")
            for nt in range(QH):
                for kc in range(DC):
                    nc.tensor.matmul(
                        st[:, nt * 512:(nt + 1) * 512],
                        lhsT=w_qk_t[:, kc, mt * 128:(mt + 1) * 128],
                        rhs=x_t[:, kc, nt * 512:(nt + 1) * 512],
                        start=(kc == 0),
                        stop=(kc == DC - 1 and not use_qkv_bias),
                    )
                if use_qkv_bias:
                    nc.tensor.matmul(
                        st[:, nt * 512:(nt + 1) * 512],
                        lhsT=qk_bias_t[:, mt * 128:(mt + 1) * 128],
                        rhs=ones_n[:, nt * 512:(nt + 1) * 512],
                        start=False, stop=True,
                    )
            with nc.allow_low_precision(reason="fp16 activations"):
                nc.scalar.copy(qk_sb[:, mt, :], st)

        def emit_v_half(x_t, v_sb, tt, nh):
            """Half v token-tile: [128tok, 512vch] psum (fl), cast ScalarE."""
            fl = ps_fl.tile([128, 512], F32, tag="fl", name="vps")
            for kc in range(DC):
                nc.tensor.matmul(
                    fl,
                    lhsT=x_t[:, kc, tt * 128:(tt + 1) * 128],
                    rhs=w_v_t[:, kc, nh * 512:(nh + 1) * 512],
                    start=(kc == 0),
                    stop=(kc == DC - 1 and not use_qkv_bias),
                )
            if use_qkv_bias:
                nc.tensor.matmul(
                    fl,
                    lhsT=ones_n[:, tt * 128:(tt + 1) * 128],
                    rhs=v_bias_t[:, nh * 512:(nh + 1) * 512],
                    start=False, stop=True,
                )
            with nc.allow_low_precision(reason="fp16 activations"):
                nc.scalar.copy(v_sb[:, tt, nh * 512:(nh + 1) * 512], fl)

        def b_phase_closures(v_sb, pt_buf, pin, rs_ps, p, rs_first, rs_last,
                             split_rs=False):
            """PV + rowsum + o-cast closures for head pair p (reads pt_buf).

            split_rs: return (main_ops, rs_ops) so the caller can front-load
            all rowsum matmuls (early reciprocal in the batch tail).
            """
            ops = []
            rs_ops = []
            for hp in range(2):
                h = 2 * p + hp
                o_tiles = {}

                def mk_alloc(qh, o_tiles=o_tiles):
                    def alloc():
                        o_tiles[qh] = ps_o.tile([128, 512], F32, tag="o",
                                                name="o_ps")
                    return alloc

                def mk_pv(qh, kc, hp=hp, h=h, o_tiles=o_tiles):
                    def pv():
                        nc.tensor.matmul(
                            o_tiles[qh],
                            lhsT=v_sb[:, kc, h * 128:(h + 1) * 128],
                            rhs=pt_buf[:, hp, kc, qh * 512:(qh + 1) * 512],
                            start=(kc == 0), stop=(kc == NT - 1),
                        )
                    return pv

                def mk_cast(qh, hp=hp, h=h, o_tiles=o_tiles):
                    def cast_o():
                        with nc.allow_low_precision(reason="fp16 unnorm O^T"):
                            nc.vector.tensor_copy(
                                pin[:, h, qh * 512:(qh + 1) * 512], o_tiles[qh])
                    return cast_o

                def mk_rs(qh, kc, hp=hp, h=h):
                    j = 2 * h + qh
                    first = rs_first and (hp == 0 and qh == 0 and kc == 0)
                    last = rs_last and (hp == 1 and qh == 1 and kc == NT - 1)

                    def rs():
                        nc.tensor.matmul(
                            rs_ps,
                            lhsT=selb_t[:, 15 - j:31 - j],
                            rhs=pt_buf[:, hp, kc, qh * 512:(qh + 1) * 512],
                            start=first, stop=last,
                            skip_group_check=True,
                        )
                    return rs

                ops.append(mk_alloc(0))
                ops.append(mk_alloc(1))
                tgt = rs_ops if split_rs else ops
                for kc in range(NT):
                    # qh-paired PVs share the V-chunk stationary operand
                    ops.append(mk_pv(0, kc))
                    ops.append(mk_pv(1, kc))
                    tgt.append(mk_rs(0, kc))
                    tgt.append(mk_rs(1, kc))
                ops.append(mk_cast(0))
                ops.append(mk_cast(1))
            if split_rs:
                return ops, rs_ops
            return ops

        def load_bias_block(bias_blocks, p, blk):
            """Load bias block blk (2 kc-tiles) for head pair p; 4 DMAs."""
            bb = bias_blocks[p][blk]
            if "t" in bb:
                return
            t = biasp.tile([128, 2, 2, N], F16, name="bias_t")
            for hp in range(2):
                h = 2 * p + hp
                for kcm in range(2):
                    for ps in range(2):
                        nc.sync.dma_start(
                            out=t[64 * ps:64 * (ps + 1), hp, kcm, :],
                            in_=bias_d[h, 2 * blk + kcm, 64 * ps:64 * (ps + 1), :],
                        )
            bb["t"] = t

        def emit_a_phase(qk_sb, pt_buf, bias_blocks, p, fillers,
                         lagged=None, fill_iters=NT):
            """Score phase for head pair p: row-tiled S^T -> exp -> bias-mult.

            Emission order per kc: S^T MMs, exp+mult, then filler closures
            (PE work that does NOT touch the st tag), then bias prefetch.
            fillers are spread over the first fill_iters iterations; lagged
            maps kc -> closures emitted at that iteration (tail-pair PV).
            """
            n_fill = len(fillers)
            fi = 0
            for kc in range(NT):
                sts = [ps_st.tile([128, N], F32, tag="st", name="st_ps")
                       for _ in range(2)]
                for hp in range(2):
                    for qh in range(QH):
                        h = 2 * p + hp
                        par = (h % 2) * 64
                        nc.tensor.matmul(
                            sts[hp][:, qh * 512:(qh + 1) * 512],
                            lhsT=qk_sb[par:par + 64, 4 + h // 2,
                                       kc * 128:(kc + 1) * 128],
                            rhs=qk_sb[par:par + 64, h // 2,
                                      qh * 512:(qh + 1) * 512],
                            start=True, stop=True,
                        )
                for hp in range(2):
                    e_t = ep.tile([128, N], F16, name="e_t")
                    with nc.allow_low_precision(reason="fp16 exp"):
                        nc.scalar.activation(
                            e_t, sts[hp],
                            mybir.ActivationFunctionType.Exp, scale=SCALE)
                    with nc.allow_low_precision(reason="fp16 P^T"):
                        nc.vector.tensor_tensor(
                            pt_buf[:, hp, kc, :], e_t,
                            bias_blocks[p][kc // 2]["t"][:, hp, kc % 2, :],
                            op=mybir.AluOpType.mult)
                ki = min(kc + 1, fill_iters)
                take = (n_fill * ki) // fill_iters - (n_fill * min(kc, fill_iters)) // fill_iters
                for _ in range(take):
                    fillers[fi]()
                    fi += 1
                if lagged is not None:
                    for op in lagged.get(kc, ()):
                        op()
                # alternating bias prefetch, 4-iteration lead: b2@kc0,
                # b3@kc2, next pair's b0@kc4, b1@kc6
                if kc % 2 == 0:
                    blk = kc // 2 + 2
                    if blk < NT // 2:
                        load_bias_block(bias_blocks, p, blk)
                    elif p + 1 < NPAIR:
                        load_bias_block(bias_blocks, p + 1, blk - NT // 2)
            assert fi == n_fill, (fi, n_fill)

        def tail_pair_ops(b, v_sb, pt_buf, pin, rs_ps, p, rs_last):
            """Last head pair of a batch: h_even streams inside A(p) via
            `lagged` (1-iteration lag behind the bias-mults); h_odd + its
            rowsums + recip + remaining casts drain densely afterwards."""
            lagged = {}
            post = []
            o_tiles = {}

            def mk_pv(hp, qh, kc):
                h = 2 * p + hp

                def pv():
                    nc.tensor.matmul(
                        o_tiles[(hp, qh)],
                        lhsT=v_sb[:, kc, h * 128:(h + 1) * 128],
                        rhs=pt_buf[:, hp, kc, qh * 512:(qh + 1) * 512],
                        start=(kc == 0), stop=(kc == NT - 1),
                    )
                return pv

            def mk_rs(hp, qh, kc):
                h = 2 * p + hp
                j = 2 * h + qh
                last = rs_last and (hp == 1 and qh == 1 and kc == NT - 1)

                def rs():
                    nc.tensor.matmul(
                        rs_ps,
                        lhsT=selb_t[:, 15 - j:31 - j],
                        rhs=pt_buf[:, hp, kc, qh * 512:(qh + 1) * 512],
                        start=False, stop=last,
                        skip_group_check=True,
                    )
                return rs

            def mk_cast(hp, qh):
                h = 2 * p + hp

                def cast_o():
                    with nc.allow_low_precision(reason="fp16 unnorm O^T"):
                        nc.vector.tensor_copy(
                            pin[:, h, qh * 512:(qh + 1) * 512],
                            o_tiles[(hp, qh)])
                return cast_o

            def mk_alloc(hp, qh):
                def alloc():
                    o_tiles[(hp, qh)] = ps_o.tile([128, 512], F32, tag="o",
                                                  name="o_ps")
                return alloc

            # h_even (hp=0): both q-halves stream per kc, 1-iteration lag,
            # starting at iteration 4 (B(p-1) fillers occupy iters 0-3)
            for kc in range(NT):
                it = max(kc + 1, 4)
                ops = lagged.setdefault(min(it, NT - 1), [])
                if kc == 0:
                    ops.append(mk_alloc(0, 0))
                    ops.append(mk_alloc(0, 1))
                ops.append(mk_pv(0, 0, kc))
                ops.append(mk_pv(0, 1, kc))
                ops.append(mk_rs(0, 0, kc))
                ops.append(mk_rs(0, 1, kc))
            # post: finish h_even casts, then h_odd rowsums (-> early recip
            # overlapping h_odd PV), then h_odd PV groups + casts
            post.append(mk_cast(0, 0))
            post.append(mk_cast(0, 1))
            for kc in range(NT):
                post.append(mk_rs(1, 0, kc))
                post.append(mk_rs(1, 1, kc))
            post.append(lambda: emit_recip_chain(b, rs_ps))
            post.append(mk_alloc(1, 0))
            post.append(mk_alloc(1, 1))
            for kc in range(NT):
                post.append(mk_pv(1, 0, kc))
                post.append(mk_pv(1, 1, kc))
            post.append(mk_cast(1, 0))
            post.append(mk_cast(1, 1))
            return lagged, post

        def emit_recip_chain(b, rs_ps):
            inv_t = invp.tile([16, 512], F16, name="inv_t")
            with nc.allow_low_precision(reason="fp16 softmax inv-denominators"):
                nc.vector.reciprocal(inv_t, rs_ps)
            nc.sync.dma_start(out=inv_scr[b], in_=inv_t)

        def norm_closures(b, pin):
            """Normalize O^T by inv denominators. The [1,1024] inv row is
            fetched to partition 0 (2KB DMA) and broadcast across the 128
            partitions with a K=1 ones-column matmul into PSUM -- no bulk
            DMA, and the PE stays warm through the epilogue."""
            ops = []

            def bcast_and_norm(h):
                bcrow = bcrp.tile([1, N], F16, name="bcrow")
                row_src = bass.AP(
                    tensor=inv_scr.tensor,
                    offset=inv_scr.offset + (b * 16 * 512) + h * N,
                    ap=[[0, 1], [1, N]],
                )
                nc.sync.dma_start(out=bcrow, in_=row_src)
                for qh in range(QH):
                    bc_ps = ps_o.tile([128, 512], F32, tag="o", name="bc_ps")
                    nc.tensor.matmul(
                        bc_ps,
                        lhsT=ones_t[0:1, 0:128],
                        rhs=bcrow[0:1, qh * 512:(qh + 1) * 512],
                        start=True, stop=True,
                    )
                    with nc.allow_low_precision(reason="fp16 normalized O^T"):
                        nc.vector.tensor_tensor(
                            pin[:, h, qh * 512:(qh + 1) * 512],
                            pin[:, h, qh * 512:(qh + 1) * 512],
                            bc_ps, op=mybir.AluOpType.mult)

            for h in range(H):
                ops.append(lambda h=h: bcast_and_norm(h))
            return ops

        def proj_mm_closures(b, pin):
            """Proj matmuls + out DMA (needs all norms done)."""
            ops = []
            for qt in range(NT):
                pp_tile = {}

                def alloc_pp(pp_tile=pp_tile):
                    pp_tile["t"] = ps_fl.tile([128, 512], F32, tag="fl",
                                              name="pp_ps")

                def proj_mms(qt=qt, pp_tile=pp_tile):
                    pp = pp_tile["t"]
                    for h in range(H):
                        last = (h == H - 1)
                        nc.tensor.matmul(
                            pp,
                            lhsT=pin[:, h, qt * 128:(qt + 1) * 128],
                            rhs=w_proj_t[:, h, :],
                            start=(h == 0),
                            stop=(last and not use_proj_bias),
                        )
                    if use_proj_bias:
                        nc.tensor.matmul(
                            pp,
                            lhsT=ones_row,
                            rhs=proj_bias_t,
                            start=False, stop=True,
                        )

                def flush_pp(qt=qt, pp_tile=pp_tile):
                    ot = outp.tile([128, 512], F32, name="ot")
                    nc.vector.tensor_copy(ot, pp_tile["t"])
                    # 4 row-split DMAs (contiguous 2KB rows, 4 queues)
                    for rp in range(4):
                        nc.sync.dma_start(
                            out=out_d[b, qt * 128 + rp * 32:
                                      qt * 128 + (rp + 1) * 32, :],
                            in_=ot[rp * 32:(rp + 1) * 32, :],
                        )

                ops.append(alloc_pp)
                ops.append(proj_mms)
                ops.append(flush_pp)
            return ops

        # =============== main emission ===============

        prev_tail = []  # closures pending from previous batch (B3 + recip)
        prev_pin = None
        for b in range(B_LOC):
            x_t = x_ts[b]
            qk_sb = qkp.tile([128, NT, N], F16, name="qk_sb")
            v_sb = vp.tile([128, NT, N], F16, name="v_sb")
            pin = pinp.tile([128, H, N], F16, name="pin")
            rs_ps = ps_rs.tile([16, 512], F32, tag="rs", name="rs_ps")
            pt_bufs = [ptp.tile([128, 2, NT, N], F16, name=f"pt{i}",
                                tag=f"pt{i}", bufs=1)
                       for i in range(2)]
            bias_blocks = [[{} for _ in range(NT // 2)] for _ in range(NPAIR)]

            # qk m-tiles (pair-0 deps first), zipped with prev batch tail
            qk_order = [0, 4, 1, 5, 2, 6, 3, 7]
            nqk = len(qk_order)
            ntail = len(prev_tail)
            ti = 0
            for i, mt in enumerate(qk_order):
                emit_qk_tile(x_t, qk_sb, mt)
                take = (ntail * (i + 1)) // nqk - (ntail * i) // nqk
                for _ in range(take):
                    prev_tail[ti]()
                    ti += 1
            assert ti == ntail
            prev_tail = []

            load_bias_block(bias_blocks, 0, 0)
            load_bias_block(bias_blocks, 0, 1)

            # A(0) fillers: v-halves (PE) + norms of b-1 (Vector/DMA)
            fillers = [
                (lambda tt=tt, nh=nh: emit_v_half(x_t, v_sb, tt, nh))
                for tt in range(NT) for nh in range(QH)
            ]
            if b > 0:
                fillers = fillers + norm_closures(b - 1, prev_pin)
            emit_a_phase(qk_sb, pt_bufs[0], bias_blocks, 0, fillers)

            for p in range(1, NPAIR):
                bops = b_phase_closures(
                    v_sb, pt_bufs[(p - 1) % 2], pin, rs_ps, p - 1,
                    rs_first=(p - 1 == 0), rs_last=False)
                if p == 1 and b > 0:
                    # proj(b-1) MMs read pin(b-1): MUST precede the o-casts
                    # of batch b (which overwrite the pin slot)
                    bops = proj_mm_closures(b - 1, prev_pin) + bops
                if p == NPAIR - 1:
                    lagged, tail_post = tail_pair_ops(
                        b, v_sb, pt_bufs[p % 2], pin, rs_ps, p, rs_last=True)
                    emit_a_phase(qk_sb, pt_bufs[p % 2], bias_blocks, p, bops,
                                 lagged=lagged, fill_iters=4)
                else:
                    emit_a_phase(qk_sb, pt_bufs[p % 2], bias_blocks, p, bops)

            prev_tail = tail_post
            prev_pin = pin

        # final tail: B3(last), recip(last), proj(last)
        for op in prev_tail:
            op()
        for op in norm_closures(B_LOC - 1, prev_pin):
            op()
        for op in proj_mm_closures(B_LOC - 1, prev_pin):
            op()

    nc.compile()
    return nc


def _prep_core_inputs(x, qkv_w, qkv_b, proj_w, proj_b, attn_biases, bias_idxs):
    """Host-side layout preparation. Returns (shared, per_core_xT, flags)."""
    x = np.ascontiguousarray(np.asarray(x, np.float32))
    qkv_w = np.asarray(qkv_w, np.float32)
    qkv_b = np.asarray(qkv_b, np.float32)
    proj_w = np.asarray(proj_w, np.float32)
    proj_b = np.asarray(proj_b, np.float32)
    attn_biases = np.asarray(attn_biases, np.float32)
    bias_idxs = np.asarray(bias_idxs)

    # qkv_w columns: per head 256 = [q 64 | k 64 | v 128]
    Wh = qkv_w.reshape(D, H, 256)
    w_q = Wh[:, :, :DK].reshape(D, H * DK)
    w_k = Wh[:, :, DK:2 * DK].reshape(D, H * DK)
    w_qk = np.concatenate([w_q, w_k], axis=1)          # [512, 1024]
    w_v = Wh[:, :, 2 * DK:].reshape(D, H * DV)         # [512, 1024]

    bh = qkv_b.reshape(H, 256)
    qk_bias = np.concatenate([bh[:, :DK].reshape(-1), bh[:, DK:2 * DK].reshape(-1)])
    v_bias = bh[:, 2 * DK:].reshape(-1)

    # exp of gathered bias, transposed to [H, k, q], tiled [H, NT, 128, N]
    BT = np.ascontiguousarray(
        np.exp(attn_biases[:, bias_idxs]).transpose(0, 2, 1))
    bias = BT.reshape(H, NT, 128, N).astype(np.float16)

    # sliding ones-column selector: col 15 = RS_SCALE, rest 0
    selb = np.zeros((128, 31), np.float16)
    selb[:, 15] = RS_SCALE

    shared = {
        "ones": np.ones((128, N), np.float16),
        "selb": selb,
        "w_qk": np.ascontiguousarray(w_qk.reshape(DC, 128, H * DK * 2)).astype(np.float16),
        "w_v": np.ascontiguousarray(w_v.reshape(DC, 128, H * DV)).astype(np.float16),
        "bias": bias,
        # 1/256 here cancels the RS_SCALE in the rowsum selector
        "w_proj": np.ascontiguousarray(
            (proj_w * (1.0 / 256.0)).reshape(H, 128, D)).astype(np.float16),
    }
    use_qkv_bias = bool(np.any(qkv_b))
    use_proj_bias = bool(np.any(proj_b))
    if use_qkv_bias:
        shared["qk_bias"] = qk_bias.reshape(1, N).astype(np.float16)
        shared["v_bias"] = v_bias.reshape(1, N).astype(np.float16)
    if use_proj_bias:
        shared["proj_bias"] = proj_b.reshape(1, D).astype(np.float16)

    xT = np.ascontiguousarray(x.transpose(0, 2, 1)).reshape(B, DC, 128, N)
    xT = xT.astype(np.float16)
    per_core = [xT[c * B_LOC:(c + 1) * B_LOC] for c in range(N_CORES)]
    return shared, per_core, use_qkv_bias, use_proj_bias


def kernel(x, qkv_w, qkv_b, proj_w, proj_b, attn_biases, bias_idxs):
    global LAST_RESULT
    shared, per_core, use_qkv_bias, use_proj_bias = _prep_core_inputs(
        x, qkv_w, qkv_b, proj_w, proj_b, attn_biases, bias_idxs)

    nc = build_program(use_qkv_bias, use_proj_bias)

    in_maps = [dict(shared, xT=per_core[c]) for c in range(N_CORES)]
    trace = bool(os.environ.get("BASS_TRACE"))
    res = run_bass_kernel_spmd(nc, in_maps, core_ids=list(range(N_CORES)),
                               trace=trace)
    LAST_RESULT = res
    out = np.concatenate([res.results[c]["out"] for c in range(N_CORES)], axis=0)
    return np.ascontiguousarray(out.astype(np.float32))
